# revision 13
# baseline (speedup 1.0000x reference)
"""BTSNet sampling kernel for 8 TRN2 NeuronCores.

Sharding: core c handles batch b=c//2, point-half h=c%2 (32768 points).
Point i of a half lives at plane position (partition i%128, slot i//128).
"""
import numpy as np

import concourse.bass as bass
import concourse.tile as tile
from concourse import bacc, mybir
from concourse.bass_utils import run_bass_kernel_spmd
from concourse.masks import make_identity

F32 = mybir.dt.float32
I32 = mybir.dt.int32
I16 = mybir.dt.int16
U8 = mybir.dt.uint8
ALU = mybir.AluOpType
ACT = mybir.ActivationFunctionType

B, P = 4, 65536
C, H, W = 64, 192, 640
PC = P // 2      # 32768 points per core
NJ = PC // 128   # 256 plane slots
NBLK = 16        # blocks of 2048 points
TPB = 2          # 1024-pt gather tiles per block
NT = NBLK * TPB  # 32 gather tiles
BS = PC // NBLK // 128   # 16 mega slots per block
GT = 1024        # points per gather (descriptor-ring limit)
KENT = H * (W // 4)        # 30720 color entries per view
FENT = H * W               # 122880 feature pair entries
EPS = 1e-3
D_MIN, D_MAX = 3.0, 80.0
A_Z = 1.0 / D_MIN - 1.0 / D_MAX
ZC_S = 2.0 / A_Z
ZC_B = -2.0 / (D_MAX * A_Z) - 1.0
HPI = 1.5707963267948966
PI = 3.141592653589793

_CACHE = {}


def build_nc():
    nc = bacc.Bacc("TRN2", target_bir_lowering=False, debug=False,
                   enable_asserts=True, num_devices=8)
    fpair = nc.dram_tensor("fpair", (FENT + 8, 128), F32, kind="ExternalInput")
    cpack = nc.dram_tensor("cpack", (4 * KENT + 1, 64), F32, kind="ExternalInput")
    xyzp = nc.dram_tensor("xyzp", (3, 128, NJ), F32, kind="ExternalInput")
    ks5 = nc.dram_tensor("ks5", (5, 3, 3), F32, kind="ExternalInput")
    ps5 = nc.dram_tensor("ps5", (5, 4, 4), F32, kind="ExternalInput")
    w1 = nc.dram_tensor("w1", (103, 128), F32, kind="ExternalInput")
    b1 = nc.dram_tensor("b1", (128, 1), F32, kind="ExternalInput")
    w2 = nc.dram_tensor("w2", (128, 1), F32, kind="ExternalInput")
    b2 = nc.dram_tensor("b2", (1, 1), F32, kind="ExternalInput")
    emp = nc.dram_tensor("emp", (64,), F32, kind="ExternalInput")

    rgbo = nc.dram_tensor("rgbo", (128, NJ, 12), F32, kind="ExternalOutput")
    invo = nc.dram_tensor("invo", (128, NJ, 4), F32, kind="ExternalOutput")
    sigo = nc.dram_tensor("sigo", (64, 512), F32, kind="ExternalOutput")

    dram_m = nc.dram_tensor("dram_m", (3, 20), F32, kind="Internal")
    dram_ix = nc.dram_tensor("dram_ix", (8, 128, NJ), I16, kind="Internal")

    with tile.TileContext(nc) as tc:
        kernel_body(tc, fpair, cpack, xyzp, ks5, ps5, w1, b1, w2, b2, emp,
                    rgbo, invo, sigo, dram_m, dram_ix)
    nc.compile()
    return nc


def kernel_body(tc, fpair, cpack, xyzp, ks5, ps5, w1, b1, w2, b2, emp,
                rgbo, invo, sigo, dram_m, dram_ix):
    nc = tc.nc
    from contextlib import ExitStack
    ctx = ExitStack()
    cpool = ctx.enter_context(tc.tile_pool(name="cpool", bufs=1))
    ppool = ctx.enter_context(tc.tile_pool(name="ppool", bufs=1))
    gpool = ctx.enter_context(tc.tile_pool(name="gpool", bufs=1))
    fpool = ctx.enter_context(tc.tile_pool(name="fpool", bufs=1))
    wpool = ctx.enter_context(tc.tile_pool(name="wpool", bufs=2))
    tpool = ctx.enter_context(tc.tile_pool(name="tpool", bufs=1))
    pspool = ctx.enter_context(tc.tile_pool(name="pspool", bufs=2, space="PSUM"))

    def tt(out, i0, i1, op):
        nc.vector.tensor_tensor(out, i0, i1, op)

    def ts1(out, i0, s1, op0):
        nc.vector.tensor_scalar(out, i0, s1, None, op0=op0)

    def ts2(out, i0, s1, s2, op0=ALU.mult, op1=ALU.add):
        nc.vector.tensor_scalar(out, i0, s1, s2, op0=op0, op1=op1)

    def stt(out, i0, s, i1, op0=ALU.mult, op1=ALU.add):
        nc.vector.scalar_tensor_tensor(out, i0, s, i1, op0=op0, op1=op1)

    def col1(ap2d):
        return ap2d.rearrange("p (n o) -> p n o", o=1)

    # ---------- constants / weights ----------
    w1sb = cpool.tile([103, 128], F32)
    nc.sync.dma_start(w1sb[:], w1.ap())
    b1sb = cpool.tile([128, 1], F32)
    nc.sync.dma_start(b1sb[:], b1.ap())
    w2sb = cpool.tile([128, 1], F32)
    nc.sync.dma_start(w2sb[:], w2.ap())
    b2sb = cpool.tile([64, 1], F32)
    nc.sync.dma_start(b2sb[:], bass.AP(b2, 0, [[0, 64], [1, 1]]))
    empsb = cpool.tile([128, 64], F32)
    nc.sync.dma_start(empsb[:], bass.AP(emp, 0, [[0, 128], [1, 64]]))
    ident = cpool.tile([128, 128], F32)
    make_identity(nc, ident[:])

    def cst(v, nm):
        t = cpool.tile([128, 1], F32, tag=f"cst_{nm}")
        nc.vector.memset(t[:], v)
        return t[:]

    c_px = cst(0.5 * W - 0.5, "px")
    c_py = cst(0.5 * H - 0.5, "py")

    # ---------- M = K @ pose[:3,:] per view, via PE ----------
    mstage = cpool.tile([3, 20], F32)
    for v in range(5):
        kt = tpool.tile([3, 3], F32, tag="kt")
        nc.sync.dma_start(kt[:], bass.AP(ks5, v * 9, [[1, 3], [3, 3]]))
        p34 = tpool.tile([3, 4], F32, tag="p34")
        nc.sync.dma_start(p34[:], bass.AP(ps5, v * 16, [[4, 3], [1, 4]]))
        mps = pspool.tile([3, 4], F32, tag="mps", space="PSUM")
        nc.tensor.matmul(mps[:], kt[:], p34[:], start=True, stop=True)
        nc.scalar.copy(mstage[:, 4 * v:4 * v + 4], mps[:])
    nc.sync.dma_start(dram_m.ap(), mstage[:])
    mb = cpool.tile([128, 60], F32)
    nc.sync.dma_start(mb[:], bass.AP(dram_m, 0, [[0, 128], [1, 60]]))

    def m_ap(v, i, j):
        k = i * 20 + 4 * v + j
        return mb[:, k:k + 1]

    # ---------- phase A: per-point planes [128, NJ] ----------
    X = ppool.tile([128, NJ], F32)
    nc.sync.dma_start(X[:], xyzp.ap()[0])
    Y = ppool.tile([128, NJ], F32)
    nc.sync.dma_start(Y[:], xyzp.ap()[1])
    Z = ppool.tile([128, NJ], F32)
    nc.sync.dma_start(Z[:], xyzp.ap()[2])

    def proj(v, persist):
        pool = ppool if persist else tpool
        sfx = "0" if persist else "c"
        us = []
        for i in range(3):
            u = tpool.tile([128, NJ], F32, tag=f"u{i}")
            ts1(u[:], X[:], m_ap(v, i, 0), ALU.mult)
            stt(u[:], Y[:], m_ap(v, i, 1), u[:])
            stt(u[:], Z[:], m_ap(v, i, 2), u[:])
            ts1(u[:], u[:], m_ap(v, i, 3), ALU.add)
            us.append(u)
        u0, u1, u2 = us
        zp = tpool.tile([128, NJ], F32, tag="ax")
        ts1(zp[:], u2[:], EPS, ALU.max)
        r = pool.tile([128, NJ], F32, tag=f"r{sfx}")
        nc.vector.reciprocal(r[:], zp[:])
        xi = pool.tile([128, NJ], F32, tag=f"xi{sfx}")
        tt(xi[:], u0[:], r[:], ALU.mult)
        yi = pool.tile([128, NJ], F32, tag=f"yi{sfx}")
        tt(yi[:], u1[:], r[:], ALU.mult)
        return xi, yi, r, u2

    def invalid_plane(xi, yi, u2):
        ax = tpool.tile([128, NJ], F32, tag="ax")
        nc.scalar.activation(ax[:], xi[:], ACT.Abs)
        ay = tpool.tile([128, NJ], F32, tag="ay")
        nc.scalar.activation(ay[:], yi[:], ACT.Abs)
        tt(ax[:], ax[:], ay[:], ALU.max)
        big = tpool.tile([128, NJ], F32, tag="u0")
        ts1(big[:], ax[:], 1.0, ALU.is_gt)
        zle = tpool.tile([128, NJ], F32, tag="u1")
        ts1(zle[:], u2[:], EPS, ALU.is_le)
        tt(big[:], big[:], zle[:], ALU.max)
        return big

    def floorp(px, nm):
        i0 = tpool.tile([128, NJ], I32, tag="cvt_i")
        nc.vector.tensor_copy(i0[:], px[:])
        f0 = tpool.tile([128, NJ], F32, tag=f"fl_{nm}")
        nc.vector.tensor_copy(f0[:], i0[:])
        gt = tpool.tile([128, NJ], F32, tag="fl_g")
        tt(gt[:], f0[:], px[:], ALU.is_gt)
        tt(f0[:], f0[:], gt[:], ALU.subtract)
        wx = tpool.tile([128, NJ], F32, tag=f"fl_w{nm}")
        tt(wx[:], px[:], f0[:], ALU.subtract)
        return f0, wx

    def pixco(xi, yi):
        px = tpool.tile([128, NJ], F32, tag="px")
        nc.scalar.activation(px[:], xi[:], ACT.Relu, bias=c_px, scale=0.5 * W)
        ts1(px[:], px[:], float(W - 1), ALU.min)
        py = tpool.tile([128, NJ], F32, tag="py")
        nc.scalar.activation(py[:], yi[:], ACT.Relu, bias=c_py, scale=0.5 * H)
        ts1(py[:], py[:], float(H - 1), ALU.min)
        x0, wx = floorp(px, "x")
        y0, wy = floorp(py, "y")
        return x0, wx, y0, wy

    def floor4(t, nm):
        q = tpool.tile([128, NJ], F32, tag="f4q")
        ts1(q[:], t[:], 0.25, ALU.mult)
        qi = tpool.tile([128, NJ], I32, tag="cvt_i")
        nc.vector.tensor_copy(qi[:], q[:])
        qf = tpool.tile([128, NJ], F32, tag=f"f4f_{nm}")
        nc.vector.tensor_copy(qf[:], qi[:])
        g = tpool.tile([128, NJ], F32, tag="fl_g")
        tt(g[:], qf[:], q[:], ALU.is_gt)
        tt(qf[:], qf[:], g[:], ALU.subtract)
        rm = tpool.tile([128, NJ], F32, tag=f"f4r_{nm}")
        stt(rm[:], qf[:], -4.0, t[:])
        return qf, rm

    # ----- feature view (v=0) -----
    xi0, yi0, r0, u20 = proj(0, True)
    inv_f = ppool.tile([128, NJ], F32, tag="inv_f")
    nc.vector.tensor_copy(inv_f[:], invalid_plane(xi0, yi0, u20)[:])
    inv_u8 = ppool.tile([128, NJ], U8, tag="inv_u8")
    nc.vector.tensor_copy(inv_u8[:], inv_f[:])
    x0f, wx0, y0f, wy0 = pixco(xi0, yi0)
    iwx0 = tpool.tile([128, NJ], F32, tag="iwx")
    nc.scalar.activation(iwx0[:], wx0[:], ACT.Copy, bias=1.0, scale=-1.0)
    iwy0 = tpool.tile([128, NJ], F32, tag="iwy")
    nc.scalar.activation(iwy0[:], wy0[:], ACT.Copy, bias=1.0, scale=-1.0)
    wq = []
    for (nm, a, b_) in (("w00", iwx0, iwy0), ("w01", iwx0, wy0),
                        ("w10", wx0, iwy0), ("w11", wx0, wy0)):
        t = ppool.tile([128, NJ], F32, tag=nm)
        tt(t[:], a[:], b_[:], ALU.mult)
        wq.append(t)
    Fp = tpool.tile([128, NJ], F32, tag="Fp")
    stt(Fp[:], y0f[:], float(W), x0f[:])
    fq, frm = floor4(Fp, "F")
    # split masks for the merge-select tree
    mq0 = ppool.tile([128, NJ], U8, tag="mq0")
    ts1(mq0[:], frm[:], 0.0, ALU.is_equal)
    mq2 = ppool.tile([128, NJ], U8, tag="mq2")
    ts1(mq2[:], frm[:], 2.0, ALU.is_equal)
    mq01 = ppool.tile([128, NJ], U8, tag="mq01")
    ts1(mq01[:], frm[:], 1.5, ALU.is_le)
    negone = cpool.tile([128, NJ], F32, tag="negone")
    nc.vector.memset(negone[:], -1.0)

    def stage_idx(pl, L):
        p32 = tpool.tile([128, NJ], I32, tag="pi32", name=f"pi32_{L}")
        nc.vector.tensor_copy(p32[:], pl)
        p16 = tpool.tile([128, NJ], I16, tag="pi16", name=f"pi16_{L}")
        nc.vector.tensor_copy(p16[:], p32[:])
        nc.sync.dma_start(dram_ix.ap()[L], p16[:])

    for q in range(4):
        msk = tpool.tile([128, NJ], U8, tag="qmsk")
        ts1(msk[:], frm[:], float(q), ALU.is_equal)
        fi = tpool.tile([128, NJ], F32, tag="fidx")
        nc.vector.select(fi[:], msk[:], fq[:], negone[:])
        stage_idx(fi[:], q)

    # ----- color views (v=1..4) -----
    invmega = ppool.tile([128, NJ, 4], F32)
    wmega = ppool.tile([128, NJ, 20], F32)
    col_iwy, col_wy = [], []
    for v in range(1, 5):
        xiv, yiv, rv, u2v = proj(v, False)
        invv = invalid_plane(xiv, yiv, u2v)
        iv_o = tpool.tile([128, NJ], F32, tag="iv_o")
        tt(iv_o[:], invv[:], inv_f[:], ALU.max)
        nc.vector.tensor_copy(invmega[:, :, v - 1:v], col1(iv_o[:]))
        x0v, wxv, y0v, wyv = pixco(xiv, yiv)
        iwxv = tpool.tile([128, NJ], F32, tag="iwx")
        nc.scalar.activation(iwxv[:], wxv[:], ACT.Copy, bias=1.0, scale=-1.0)
        iwyv = ppool.tile([128, NJ], F32, tag=f"iwy{v}")
        nc.scalar.activation(iwyv[:], wyv[:], ACT.Copy, bias=1.0, scale=-1.0)
        wyp = ppool.tile([128, NJ], F32, tag=f"wy{v}")
        nc.vector.tensor_copy(wyp[:], wyv[:])
        col_iwy.append(iwyv)
        col_wy.append(wyp)
        kf, rmv = floor4(x0v, "k")
        ci = tpool.tile([128, NJ], F32, tag="cidx")
        stt(ci[:], y0v[:], float(W // 4), kf[:])
        stage_idx(ci[:], 4 + v - 1)
        eqs = []
        for pxi in range(4):
            e = tpool.tile([128, NJ], F32, tag=f"eq{pxi}")
            ts1(e[:], rmv[:], float(pxi), ALU.is_equal)
            eqs.append(e)
        wv = wmega[:, :, (v - 1) * 5:(v - 1) * 5 + 5]
        t0 = tpool.tile([128, NJ], F32, tag="wtmp0")
        tt(t0[:], iwxv[:], eqs[0][:], ALU.mult)
        nc.vector.tensor_copy(wv[:, :, 0:1], col1(t0[:]))
        for pxi in range(1, 4):
            ta = tpool.tile([128, NJ], F32, tag="wtmpa")
            tt(ta[:], iwxv[:], eqs[pxi][:], ALU.mult)
            tb = tpool.tile([128, NJ], F32, tag="wtmpb")
            tt(tb[:], wxv[:], eqs[pxi - 1][:], ALU.mult)
            tt(ta[:], ta[:], tb[:], ALU.add)
            nc.vector.tensor_copy(wv[:, :, pxi:pxi + 1], col1(ta[:]))
        t4 = tpool.tile([128, NJ], F32, tag="wtmp4")
        tt(t4[:], wxv[:], eqs[3][:], ALU.mult)
        nc.vector.tensor_copy(wv[:, :, 4:5], col1(t4[:]))

    # ----- wrapped idx readback -----
    wrapped = []
    for L in range(8):
        wr = ppool.tile([128, NT * 64], I16, tag=f"wr{L}")
        for a in range(8):
            nc.sync.dma_start(
                wr[16 * a:16 * a + 16, :].rearrange("q (m a2) -> q m a2", a2=8),
                bass.AP(dram_ix, L * 128 * NJ, [[NJ, 16], [1, NJ], [16 * NJ, 8]]))
        wrapped.append(wr)

    fin = [bass.AP(fpair, q * 128, [[512, FENT // 4], [1, 256]])
           for q in range(4)]
    cin = [bass.AP(cpack, (v * KENT) * 64, [[64, KENT], [1, 64]])
           for v in range(4)]

    # ---------- phase B ----------
    sigpre = ppool.tile([64, 512], F32)
    gsem = [nc.alloc_semaphore(f"gs{i}") for i in range(NT)]

    for blk in range(NBLK):
        bs = slice(blk * BS, blk * BS + BS)
        cdest = [gpool.tile([128, BS, 64], F32, tag=f"cd{v}", name=f"cd{v}_{blk}") for v in range(4)]
        mlpmega = gpool.tile([128, BS, 103], F32, tag="mlpmega", name=f"mlpmega_{blk}")
        for t in range(TPB):
            gtile = blk * TPB + t
            fdq = [fpool.tile([128, 8, 256], F32, tag=f"fd{q}",
                              name=f"fd{q}_{blk}_{t}") for q in range(4)]
            with tc.tile_critical():
                k = 0
                for q in range(4):
                    k += 1
                    nc.gpsimd.dma_gather(
                        out_ap=fdq[q][:], in_ap=fin[q],
                        idxs_ap=wrapped[q][:, gtile * 64:(gtile + 1) * 64],
                        num_idxs=GT, num_idxs_reg=GT, elem_size=256,
                        elem_step=512).then_inc(gsem[gtile], 16)
                    nc.gpsimd.wait_ge(gsem[gtile], 16 * k)
                for v in range(4):
                    k += 1
                    nc.gpsimd.dma_gather(
                        out_ap=cdest[v][:, t * 8:(t + 1) * 8, :], in_ap=cin[v],
                        idxs_ap=wrapped[4 + v][:, gtile * 64:(gtile + 1) * 64],
                        num_idxs=GT, num_idxs_reg=GT, elem_size=64,
                        elem_step=64).then_inc(gsem[gtile], 16)
                    nc.gpsimd.wait_ge(gsem[gtile], 16 * k)
            # merge the 4 feature splits via select tree (outside critical)
            js = slice(gtile * 8, gtile * 8 + 8)

            def mb8(mask):
                return col1(mask[:, js]).to_broadcast([128, 8, 256])
            nc.vector.select(fdq[1][:], mb8(mq0), fdq[0][:], fdq[1][:])
            nc.vector.select(fdq[3][:], mb8(mq2), fdq[2][:], fdq[3][:])
            fsum = fdq[3]
            nc.vector.select(fsum[:], mb8(mq01), fdq[1][:], fdq[3][:])
            gv = fsum[:].rearrange("p n (x y c) -> p n x y c", x=2, y=2)

            def wb(wt):
                return col1(wt[:, js]).to_broadcast([128, 8, 64])
            acc = fpool.tile([128, 8, 64], F32, tag="facc",
                             name=f"facc_{blk}_{t}")
            tt(acc[:], gv[:, :, 0, 0, :], wb(wq[0]), ALU.mult)
            tm = fpool.tile([128, 8, 64], F32, tag="ftmp",
                            name=f"ftmp_{blk}_{t}")
            tt(tm[:], gv[:, :, 0, 1, :], wb(wq[1]), ALU.mult)
            tt(acc[:], acc[:], tm[:], ALU.add)
            tt(tm[:], gv[:, :, 1, 0, :], wb(wq[2]), ALU.mult)
            tt(acc[:], acc[:], tm[:], ALU.add)
            tt(tm[:], gv[:, :, 1, 1, :], wb(wq[3]), ALU.mult)
            tt(acc[:], acc[:], tm[:], ALU.add)
            ms = slice(t * 8, t * 8 + 8)
            nc.vector.select(
                mlpmega[:, ms, 0:64],
                col1(inv_u8[:, js]).to_broadcast([128, 8, 64]),
                empsb[:].rearrange("p (o c) -> p o c", o=1)
                    .to_broadcast([128, 8, 64]),
                acc[:])

        # ----- posenc into mlpmega[..., 64:103] -----
        co = 64

        def mslot(k):
            return mlpmega[:, :, co + k:co + k + 1]
        nc.scalar.copy(mslot(0), col1(xi0[:, bs]))
        nc.scalar.copy(mslot(1), col1(yi0[:, bs]))
        nc.scalar.activation(mslot(2), col1(r0[:, bs]), ACT.Copy,
                             bias=ZC_B, scale=ZC_S)
        for ci_ in range(3):
            src = mslot(ci_)
            nc.scalar.activation(mslot(3 + ci_), src, ACT.Sin, scale=1.5)
            xb = tpool.tile([128, BS], F32, tag="xb")
            nc.vector.tensor_copy(xb[:], src.rearrange("p n o -> p (n o)"))
            ts1(xb[:], xb[:], 1.5, ALU.mult)
            wrp = tpool.tile([128, BS], F32, tag="wrp")
            nc.vector.add_range_wrap(wrp[:], xb[:], shift=HPI, bound=PI,
                                     period=2 * PI)
            nc.scalar.activation(mslot(6 + ci_), col1(wrp[:]), ACT.Sin)
        for f in range(1, 6):
            for ci_ in range(3):
                sp = mslot(3 + 6 * (f - 1) + ci_)
                cp = mslot(6 + 6 * (f - 1) + ci_)
                sn = mslot(3 + 6 * f + ci_)
                cn = mslot(6 + 6 * f + ci_)
                nc.vector.scalar_tensor_tensor(sn, sp, 2.0, cp,
                                               op0=ALU.mult, op1=ALU.mult)
                t2 = tpool.tile([128, BS], F32, tag="dbl")
                tt(t2[:], sp.rearrange("p n o -> p (n o)"),
                   sp.rearrange("p n o -> p (n o)"), ALU.mult)
                ts2(t2[:], t2[:], -2.0, 1.0)
                nc.vector.tensor_copy(cn, col1(t2[:]))

        # ----- color blend -----
        rgbblk = wpool.tile([128, BS, 12], F32, tag="rgbblk")
        for v in range(4):
            g5 = cdest[v][:, :, 0:30].rearrange("p n (x y c) -> p n x y c", x=5, y=2, c=3)

            def cb(wt):
                return col1(wt[:, bs]) \
                    .rearrange("p n (o2 o3) -> p n o2 o3", o2=1) \
                    .to_broadcast([128, BS, 5, 3])
            ty = wpool.tile([128, BS, 5, 3], F32, tag="cty")
            tt(ty[:], g5[:, :, :, 0, 0:3], cb(col_iwy[v]), ALU.mult)
            ty2 = wpool.tile([128, BS, 5, 3], F32, tag="cty2")
            tt(ty2[:], g5[:, :, :, 1, 0:3], cb(col_wy[v]), ALU.mult)
            tt(ty[:], ty[:], ty2[:], ALU.add)
            wm = wmega[:, bs, 5 * v:5 * v + 5]
            racc = wpool.tile([128, BS, 3], F32, tag="racc")
            tt(racc[:], ty[:, :, 0, :],
               wm[:, :, 0:1].to_broadcast([128, BS, 3]), ALU.mult)
            rtm = wpool.tile([128, BS, 3], F32, tag="rtm")
            for pxi in range(1, 5):
                tt(rtm[:], ty[:, :, pxi, :],
                   wm[:, :, pxi:pxi + 1].to_broadcast([128, BS, 3]), ALU.mult)
                tt(racc[:], racc[:], rtm[:], ALU.add)
            nc.vector.tensor_copy(rgbblk[:, :, 3 * v:3 * v + 3], racc[:])
        nc.sync.dma_start(
            bass.AP(rgbo, blk * BS * 12, [[NJ * 12, 128], [12, BS], [1, 12]]),
            rgbblk[:])

        # ----- transposes + MLP -----
        mlpT = gpool.tile([103, BS * 128], F32, tag="mlpT")
        for j in range(BS):
            trp = pspool.tile([103, 128], F32, tag="trp", space="PSUM")
            nc.tensor.transpose(trp[:], mlpmega[:, j, :], ident[:])
            nc.scalar.copy(mlpT[:, j * 128:(j + 1) * 128], trp[:])
        for hblk in range(BS * 128 // 512):
            hs = slice(hblk * 512, (hblk + 1) * 512)
            hps = pspool.tile([128, 512], F32, tag="hps", space="PSUM")
            nc.tensor.matmul(hps[:], w1sb[:], mlpT[:, hs], start=True, stop=True)
            hrel = wpool.tile([128, 512], F32, tag="hrel")
            nc.scalar.activation(hrel[:], hps[:], ACT.Relu, bias=b1sb[:])
            sps = pspool.tile([1, 512], F32, tag="sps", space="PSUM")
            nc.tensor.matmul(sps[:], w2sb[:], hrel[:], start=True, stop=True)
            part = blk * (BS * 128 // 512) + hblk
            sstg = wpool.tile([1, 512], F32, tag="sstg",
                              name=f"sstg_{blk}_{hblk}")
            nc.scalar.copy(sstg[:], sps[:])
            nc.sync.dma_start(sigpre[part:part + 1, :], sstg[:])

    # ---------- softplus sigma ----------
    ts1(sigpre[:], sigpre[:], b2sb[:, 0:1], ALU.add)
    az = ppool.tile([64, 512], F32, tag="az")
    nc.scalar.activation(az[:], sigpre[:], ACT.Abs)
    ez = ppool.tile([64, 512], F32, tag="ez")
    nc.scalar.activation(ez[:], az[:], ACT.Exp, scale=-1.0)
    lz = ppool.tile([64, 512], F32, tag="lz")
    nc.scalar.activation(lz[:], ez[:], ACT.Ln, bias=1.0)
    rz = ppool.tile([64, 512], F32, tag="rz")
    nc.scalar.activation(rz[:], sigpre[:], ACT.Relu)
    tt(rz[:], rz[:], lz[:], ALU.add)

    # ---------- outputs ----------
    nc.sync.dma_start(invo.ap(), invmega[:])
    nc.sync.dma_start(sigo.ap(), rz[:])
    ctx.close()


def _stage_inputs(xyz, gff, gfk, gfp, gci, gck, gcp, emp, W1, b1, W2, b2):
    in_maps = []
    fpairs, cpacks, ks5s, ps5s = {}, {}, {}, {}
    for b in range(B):
        img = gff[b, 0]                       # (64, H, W)
        hwc = np.ascontiguousarray(img.transpose(1, 2, 0))   # (H, W, 64)
        nxt = hwc[np.minimum(np.arange(H) + 1, H - 1)]
        fp = np.concatenate([hwc, nxt], axis=2).reshape(H * W, 128)
        fp = np.concatenate([fp, np.zeros((8, 128), np.float32)], 0)
        fpairs[b] = np.ascontiguousarray(fp)

        cimg = gci[b]                          # (4, 3, H, W)
        xw = np.minimum(np.arange(W // 4)[:, None] * 4 + np.arange(5)[None, :],
                        W - 1)                 # (160, 5)
        yp = np.minimum(np.arange(H)[:, None] + np.arange(2)[None, :], H - 1)
        # -> (v, c, y, row, k, px): index [y,row] then [k,px]
        cp = cimg[:, :, yp[:, :, None, None], xw[None, None, :, :]]
        # dims: (v, c, y, row, k, px) -> (v, y, k, px, row, c)
        cp = cp.transpose(0, 2, 4, 5, 3, 1).reshape(4 * KENT, 30)
        cpk = np.zeros((4 * KENT + 1, 64), np.float32)
        cpk[:4 * KENT, :30] = cp
        cpacks[b] = cpk
        ks5s[b] = np.ascontiguousarray(
            np.concatenate([gfk[b], gck[b]], 0)).astype(np.float32)
        ps5s[b] = np.ascontiguousarray(
            np.concatenate([gfp[b], gcp[b]], 0)).astype(np.float32)

    for c in range(8):
        b, half = c // 2, c % 2
        pts = xyz[b, half * PC:(half + 1) * PC]            # (PC, 3)
        xyzp = np.ascontiguousarray(
            pts.T.reshape(3, NJ, 128).transpose(0, 2, 1))  # (3, 128, NJ)
        in_maps.append(dict(
            fpair=fpairs[b], cpack=cpacks[b], xyzp=xyzp,
            ks5=ks5s[b], ps5=ps5s[b],
            w1=W1.astype(np.float32), b1=b1.reshape(128, 1).astype(np.float32),
            w2=W2.astype(np.float32), b2=b2.reshape(1, 1).astype(np.float32),
            emp=emp.astype(np.float32)))
    return in_maps


def kernel(xyz, grid_f_features, grid_f_Ks, grid_f_poses_w2c, grid_c_imgs,
           grid_c_Ks, grid_c_poses_w2c, empty_feature, W1, b1, W2, b2):
    xyz = np.asarray(xyz, np.float32)
    in_maps = _stage_inputs(
        xyz, np.asarray(grid_f_features, np.float32),
        np.asarray(grid_f_Ks, np.float32), np.asarray(grid_f_poses_w2c, np.float32),
        np.asarray(grid_c_imgs, np.float32), np.asarray(grid_c_Ks, np.float32),
        np.asarray(grid_c_poses_w2c, np.float32),
        np.asarray(empty_feature, np.float32),
        np.asarray(W1, np.float32), np.asarray(b1, np.float32),
        np.asarray(W2, np.float32), np.asarray(b2, np.float32))
    if "nc" not in _CACHE:
        _CACHE["nc"] = build_nc()
    res = run_bass_kernel_spmd(_CACHE["nc"], in_maps,
                               core_ids=list(range(8))).results

    rgb = np.zeros((B, P, 12), np.float32)
    inv = np.zeros((B, P, 4), np.float32)
    sig = np.zeros((B, P, 1), np.float32)
    for c in range(8):
        b, half = c // 2, c % 2
        sl = slice(half * PC, (half + 1) * PC)
        rgb[b, sl] = res[c]["rgbo"].transpose(1, 0, 2).reshape(PC, 12)
        inv[b, sl] = res[c]["invo"].transpose(1, 0, 2).reshape(PC, 4)
        sig[b, sl, 0] = res[c]["sigo"].reshape(PC)
    return rgb, inv, sig
